# revision 26
# baseline (speedup 1.0000x reference)
"""Hamiltonian block-generation layer on 8 Trainium2 NeuronCores.

Strategy (v3, transfer-minimal): the axon tunnel (~65-145 MB/s) dominates, so
the kernel ships the minimum bytes that carry real information.

Pair sharding exploits the triu structure: pairs (i, j>i) with the same i form
a contiguous run. Core m takes rows i == m (mod 8); slot k on every core has
the same padded width W_k = 16*ceil((511-8k)/16) (sum 16896 = 132*128), so the
program is identical across cores (pure SPMD) while the data differs.

Per slot (row i), the stage-1 pre-activation  x @ Wo1  splits into
  -  e_ij  @ Wo1_e : per-pair matmul over the uploaded e^T tile,
  -  n_j   @ Wo1_n : matmul against a contiguous slice of a small per-core
                     shifted node table (n_j runs j = i+1 .. 511),
  -  n_i   @ Wo1_i + bo1 : constant per slot -> host-computed f32 bias vector
                     fed to the Silu activation (64 KB upload).
so the 33 MB of replicated per-pair node features is never transferred.

The device returns only the off-diagonal MLP delta mo, quantized to per-pair
symmetric int4 (two codes per byte, plus the per-pair scale encoded in-band as
two fixed-point u8 columns — one output tensor, since each fetched tensor pays
~0.1-0.2 s of tunnel round-trip latency). Host dequantizes, adds overlap +
bo2, writes both block orientations, and runs the tiny diagonal MLP in exact
f32 numpy. Total quantization lands ~6e-3 relative to the Hamiltonian absmax,
under the 2e-2 gate with 3x margin.

Measured on this setup: ~17 MB up + ~13.3 MB down per call, warm run+transfer
~0.7 s (baseline scheme: ~9.5 s). The persistent XLA cache below collapses
the fresh-process NEFF compile (minutes, variable) to seconds when it hits.

If pair_i/pair_j are not the standard lexicographic triu enumeration the
kernel falls back to an exact host-side computation (never triggers for the
reference's setup_inputs).
"""

import os

import numpy as np
import ml_dtypes

# Persistent XLA compilation cache: the axon runner re-jits its wrapper on
# every call (fresh closure), paying ~0.4s XLA compile per call and the full
# NEFF compile per fresh process. A disk cache turns both into fast loads.
try:
    import jax
    jax.config.update("jax_compilation_cache_dir",
                      os.path.expanduser("~/.jax_xla_cache"))
    jax.config.update("jax_persistent_cache_min_compile_time_secs", 0)
    jax.config.update("jax_persistent_cache_min_entry_size_bytes", -1)
except Exception:
    pass

F8 = ml_dtypes.float8_e4m3
BF16 = ml_dtypes.bfloat16

N_ATOMS = 512
B = 14
BB = B * B          # 196
F = 128
FE = 128
HID = 256
P = N_ATOMS * (N_ATOMS - 1) // 2   # 130816
NCORES = 8

# slot layout: core m, slot k -> row i = m + 8k, true width 511 - i,
# padded width W[k] = 4*ceil((511-8k)/4) = 512-8k  (same on every core)
_KS = np.arange(64)
W_SLOT = 512 - 8 * _KS
OFF_SLOT = np.concatenate([[0], np.cumsum(W_SLOT)])[:-1]
COLS = int(W_SLOT.sum())           # 16640 = 130*128
assert COLS % 128 == 0
NJP = 512                          # shifted node table width (= max 8k+W_k)
SK = 327675.0                      # int4 scale codec: code = scale * SK (16-bit)

_VALID_DTS = ("float8e4", "bfloat16", "float32")
IN_DT = os.environ.get("KERNEL_IN_DT", "float8e4")
OUT_DT = os.environ.get("KERNEL_OUT_DT", "int4")
if IN_DT not in _VALID_DTS:
    IN_DT = "float8e4"
if OUT_DT not in _VALID_DTS + ("int4",):
    OUT_DT = "int4"

_CACHE = {}


def _np_dt(name):
    return {"float8e4": F8, "bfloat16": BF16, "float32": np.float32}[name]


def _build_nc(in_dt_name, out_dt_name):
    import concourse.mybir as mybir
    import concourse.tile as tile
    from concourse import bacc

    f32 = mybir.dt.float32
    bf16 = mybir.dt.bfloat16
    u8 = mybir.dt.uint8
    alu = mybir.AluOpType
    in_dt = getattr(mybir.dt, in_dt_name)
    int4_out = out_dt_name == "int4"
    out_dt = None if int4_out else getattr(mybir.dt, out_dt_name)
    NT = COLS // 128                   # stage-2 tiles (130)

    nc = bacc.Bacc("TRN2", target_bir_lowering=False)

    # xall packs eT [128, COLS] | nodesJ [128, NJP] | Wo1^T chunks [128, 2*HID]
    xall = nc.dram_tensor("xall", [128, COLS + NJP + 2 * HID], in_dt,
                          kind="ExternalInput")
    # aux packs Ab biases [128, 128] | Wo2^T chunks [128, 2*BB], all bf16
    aux = nc.dram_tensor("aux", [128, 2 * 64 + 2 * BB], bf16,
                         kind="ExternalInput")
    if int4_out:
        # per pair: 98 bytes of 4-bit code pairs + 2-byte fixed-point scale
        mo = nc.dram_tensor("mo", [COLS, BB // 2 + 2], u8, kind="ExternalOutput")
    else:
        mo = nc.dram_tensor("mo", [COLS, BB], out_dt, kind="ExternalOutput")

    silu = mybir.ActivationFunctionType.Silu

    with tile.TileContext(nc) as tc:
        with tc.tile_pool(name="consts", bufs=1) as consts, \
             tc.tile_pool(name="outp", bufs=3) as outp, \
             tc.tile_pool(name="psH", bufs=2, space="PSUM") as psH, \
             tc.tile_pool(name="psO", bufs=4, space="PSUM") as psO:

            # ---- resident inputs
            xa = consts.tile([128, COLS + NJP + 2 * HID], in_dt, tag="xa")
            nc.sync.dma_start(out=xa, in_=xall[:, :])
            et = xa[:, 0:COLS]
            nj = xa[:, COLS:COLS + NJP]
            wo1 = xa[:, COLS + NJP:].rearrange("p (c h) -> p c h", c=2)
            auxt = consts.tile([128, 2 * 64 + 2 * BB], bf16, tag="auxt")
            nc.sync.dma_start(out=auxt, in_=aux[:, :])
            wo2 = auxt[:, 2 * 64:].rearrange("p (c e) -> p c e", c=2)
            abt = consts.tile([128, 2 * 64], f32, tag="abt")
            nc.vector.tensor_scalar_add(abt, auxt[:, 0:2 * 64], 0.0)
            hob = consts.tile([128, 2, COLS], bf16, tag="hob")

            # ---- stage 1: ho^T = silu(Wo1_n^T nj + Wo1_e^T e + A_i + bo1)
            for k in range(64):
                off = int(OFF_SLOT[k])
                w = int(W_SLOT[k])
                for h in range(2):
                    ph = psH.tile([128, 512], f32, tag=f"psh{h}")
                    nc.tensor.matmul(
                        ph[:, :w], wo1[:, 1, h * 128:(h + 1) * 128],
                        et[:, off:off + w], start=True, stop=False)
                    nc.tensor.matmul(
                        ph[:, :w], wo1[:, 0, h * 128:(h + 1) * 128],
                        nj[:, 8 * k:8 * k + w], start=False, stop=True)
                    nc.scalar.activation(
                        hob[:, h, off:off + w], ph[:, :w], silu,
                        bias=abt[:, h * 64 + k:h * 64 + k + 1])

            # ---- stage 2: mo = ho^T.T @ Wo2
            if int4_out:
                sclt = consts.tile([128, NT], f32, tag="sclt")
            for t in range(NT):
                ps = psO.tile([128, BB], f32, tag="pso")
                for h in range(2):
                    nc.tensor.matmul(
                        ps, hob[:, h, t * 128:(t + 1) * 128],
                        wo2[:, h, :], start=(h == 0), stop=(h == 1))
                if not int4_out:
                    ot = outp.tile([128, BB], out_dt, tag="ot")
                    nc.scalar.copy(ot, ps)
                    nc.sync.dma_start(out=mo[t * 128:(t + 1) * 128, :], in_=ot)
                    continue
                # per-pair symmetric int4: q = rhe(x*7/rowmax) + 8 in [1,15]
                rm = outp.tile([128, 1], f32, tag="rm")
                nc.vector.tensor_reduce(rm, ps, mybir.AxisListType.X, alu.max,
                                        apply_absolute_value=True)
                nc.vector.tensor_scalar_max(rm, rm, 1e-30)
                nc.vector.tensor_scalar_mul(sclt[:, t:t + 1], rm, 1.0 / 7.0)
                rinv = outp.tile([128, 1], f32, tag="rinv")
                nc.vector.reciprocal(rinv, sclt[:, t:t + 1])
                q8 = outp.tile([128, BB], u8, tag="q8")
                nc.vector.tensor_scalar(q8, ps, rinv, 8.0, alu.mult, alu.add)
                q3 = q8[:, :].rearrange("p (c two) -> p c two", two=2)
                hi = outp.tile([128, BB // 2], u8, tag="hi")
                nc.vector.tensor_scalar(hi, q3[:, :, 1], 4, None,
                                        alu.logical_shift_left)
                pk8 = outp.tile([128, BB // 2 + 2], u8, tag="pk8")
                nc.vector.tensor_tensor(pk8[:, 0:BB // 2], hi, q3[:, :, 0],
                                        alu.bitwise_or)
                # scale -> 16-bit fixed point in two u8 columns (hi, lo+127.5)
                cf = outp.tile([128, 1], f32, tag="cf")
                nc.vector.tensor_scalar(cf, sclt[:, t:t + 1], SK, None, alu.mult)
                nc.vector.tensor_scalar_min(cf, cf, 65279.0)
                nc.vector.tensor_scalar_mul(pk8[:, BB // 2:BB // 2 + 1],
                                            cf, 1.0 / 256.0)
                hif = outp.tile([128, 1], f32, tag="hif")
                nc.vector.tensor_scalar_add(hif, pk8[:, BB // 2:BB // 2 + 1], 0.0)
                lof = outp.tile([128, 1], f32, tag="lof")
                nc.vector.scalar_tensor_tensor(lof, hif, -256.0, cf,
                                               alu.mult, alu.add)
                nc.vector.tensor_scalar_add(pk8[:, BB // 2 + 1:BB // 2 + 2],
                                            lof, 127.5)
                nc.sync.dma_start(out=mo[t * 128:(t + 1) * 128, :], in_=pk8)

    nc.finalize()
    return nc


def _triu_maps():
    """Device-order <-> input-order index maps (input = lexicographic triu).

    Returns (dev_idx, inp_idx): mo_global[dev_idx] are the valid device rows,
    belonging to triu positions inp_idx.
    """
    if "maps" in _CACHE:
        return _CACHE["maps"]
    base = np.concatenate([[0], np.cumsum(511 - np.arange(512))])[:-1]  # [512]
    dev_idx = []
    inp_idx = []
    for m in range(NCORES):
        rows = m + 8 * _KS                       # [64]
        for k in range(64):
            i = int(rows[k])
            L = 511 - i
            if L <= 0:
                continue
            dev_idx.append(m * COLS + int(OFF_SLOT[k]) + np.arange(L))
            inp_idx.append(int(base[i]) + np.arange(L))
    maps = (np.concatenate(dev_idx), np.concatenate(inp_idx))
    _CACHE["maps"] = maps
    return maps


def _silu(z):
    return z / (1.0 + np.exp(-z))


def _host_fallback(nodes, edges, overlap, Wo1, bo1, Wo2, bo2, pair_i, pair_j):
    """Exact f32 off-diagonal blocks for arbitrary pair lists."""
    out = np.empty((len(pair_i), BB), np.float32)
    CH = 8192
    for s in range(0, len(pair_i), CH):
        pi = pair_i[s:s + CH]
        pj = pair_j[s:s + CH]
        x = np.concatenate(
            [nodes[pi], nodes[pj], edges[pi, pj]], axis=1)
        out[s:s + CH] = _silu(x @ Wo1 + bo1) @ Wo2 + bo2
    return overlap.reshape(-1, BB) + out


def kernel(**inputs) -> np.ndarray:
    nodes = np.ascontiguousarray(np.asarray(inputs["nodes_feature"], np.float32))
    edges = np.asarray(inputs["edges_feature"], np.float32)
    atom_blocks = np.asarray(inputs["atom_blocks"], np.float32)
    overlap = np.asarray(inputs["overlap_pair"], np.float32)
    W1 = np.asarray(inputs["W1"], np.float32)
    b1 = np.asarray(inputs["b1"], np.float32)
    W2 = np.asarray(inputs["W2"], np.float32)
    b2 = np.asarray(inputs["b2"], np.float32)
    Wo1 = np.ascontiguousarray(np.asarray(inputs["Wo1"], np.float32))
    bo1 = np.asarray(inputs["bo1"], np.float32)
    Wo2 = np.ascontiguousarray(np.asarray(inputs["Wo2"], np.float32))
    bo2 = np.asarray(inputs["bo2"], np.float32)
    pair_i = np.asarray(inputs["pair_i"]).astype(np.int64)
    pair_j = np.asarray(inputs["pair_j"]).astype(np.int64)

    tri, trj = np.triu_indices(N_ATOMS, k=1)
    is_triu = np.array_equal(pair_i, tri) and np.array_equal(pair_j, trj)

    if is_triu:
        off = _device_off_blocks(nodes, edges, overlap, Wo1, bo1, Wo2, bo2)
    else:
        off = _host_fallback(nodes, edges, overlap, Wo1, bo1, Wo2, bo2,
                             pair_i, pair_j)

    # ---- diagonal blocks (exact f32, host)
    ar = np.arange(N_ATOMS)
    xd = np.concatenate([nodes, edges[ar, ar]], axis=1)            # [512, 256]
    md = _silu(xd @ W1 + b1) @ W2 + b2
    dblk = atom_blocks + md.reshape(-1, B, B)

    # ---- scatter into dense H
    H4 = np.zeros((N_ATOMS, B, N_ATOMS, B), np.float32)
    H4[ar, :, ar, :] = dblk
    offb = off.reshape(P, B, B)
    H4[pair_i, :, pair_j, :] = offb
    H4[pair_j, :, pair_i, :] = offb.transpose(0, 2, 1)
    return H4.reshape(N_ATOMS * B, N_ATOMS * B)


def _device_off_blocks(nodes, edges, overlap, Wo1, bo1, Wo2, bo2):
    in_np = _np_dt(IN_DT)

    # ---- host prep
    nodesT_q = np.ascontiguousarray(nodes.T).astype(in_np)         # [128, 512]
    # packed Wo1^T chunks: wo1p[p, c*256 + h] = Wo1[128 + c*128 + p, h]
    wo1p = np.ascontiguousarray(
        Wo1[128:384].reshape(2, 128, HID).transpose(1, 0, 2).reshape(128, 2 * HID)
    ).astype(in_np)
    wo2p = np.ascontiguousarray(
        Wo2.reshape(2, 128, BB).transpose(1, 0, 2).reshape(128, 2 * BB)
    ).astype(BF16)
    A_all = nodes @ Wo1[:128] + bo1                                # [512, 256] f32

    in_maps = []
    ebuf = np.zeros((COLS, 128), in_np)
    for m in range(NCORES):
        rows = m + 8 * _KS
        ebuf[:] = 0
        for k in range(64):
            i = int(rows[k])
            L = 511 - i
            if L > 0:
                o = int(OFF_SLOT[k])
                ebuf[o:o + L] = edges[i, i + 1:512]
        xall = np.zeros((128, COLS + NJP + 2 * HID), in_np)
        xall[:, 0:COLS] = ebuf.T
        xall[:, COLS:COLS + 511 - m] = nodesT_q[:, m + 1:]
        xall[:, COLS + NJP:] = wo1p
        # bias table: Ab[p, h*64+k] = A_all[row_k, h*128+p]
        Ab = A_all[rows].reshape(64, 2, 128).transpose(2, 1, 0).reshape(128, 128)
        aux = np.concatenate([Ab.astype(BF16), wo2p], axis=1)
        in_maps.append({"xall": xall, "aux": np.ascontiguousarray(aux)})

    key = ("nc", IN_DT, OUT_DT)
    if key not in _CACHE:
        _CACHE[key] = _build_nc(IN_DT, OUT_DT)
    nc = _CACHE[key]

    import time
    from concourse.bass_utils import run_bass_kernel_spmd
    trace = bool(int(os.environ.get("KERNEL_TRACE", "0")))
    t0 = time.time()
    if trace:
        try:
            res = run_bass_kernel_spmd(nc, in_maps, core_ids=list(range(NCORES)),
                                       trace=True)
        except Exception:
            res = run_bass_kernel_spmd(nc, in_maps, core_ids=list(range(NCORES)))
    else:
        res = run_bass_kernel_spmd(nc, in_maps, core_ids=list(range(NCORES)))
    _CACHE["run_wall_s"] = time.time() - t0
    _CACHE["last_result"] = res

    # ---- reorder device rows into triu order, add overlap + bias
    dev_idx, inp_idx = _triu_maps()
    mo_inp = np.empty((P, BB), np.float32)
    if OUT_DT == "int4":
        pk = np.concatenate([res.results[m]["mo"] for m in range(NCORES)])
        pk_d = pk[dev_idx]
        hi = pk_d[:, BB // 2].astype(np.float32)
        lo = pk_d[:, BB // 2 + 1].astype(np.float32)
        scl_d = ((hi * 256.0 + lo - 127.5) * (1.0 / SK)).reshape(-1, 1)
        nib = pk_d[:, :BB // 2]
        q = np.empty((len(dev_idx), BB), np.float32)
        q[:, 0::2] = nib & 15
        q[:, 1::2] = nib >> 4
        q -= 8.0
        q *= scl_d
        mo_inp[inp_idx] = q
    else:
        mo_all = np.concatenate([res.results[m]["mo"] for m in range(NCORES)])
        mo_inp[inp_idx] = mo_all[dev_idx].astype(np.float32)
    return overlap.reshape(P, BB) + mo_inp + bo2
